# revision 11
# baseline (speedup 1.0000x reference)
"""Expert-parallel MoE kernel for one TRN2 chip (8 NeuronCores).

Strategy (expert-parallel, top-2 sparse):
  - core e owns expert e's weights (weight-norm applied + transposed on
    device, bf16 compute / fp32 accumulate).
  - gating is data-parallel: core i computes top-2 masks for its T/8
    token slice of x with exact fp32 matmuls (so routing never flips),
    then an AllToAll exchanges per-expert mask rows so every core holds
    the full-T mask for ITS expert.
  - token compaction: matmul prefix-sums turn the mask into global
    compacted positions; x rows (bf16) are scatter-written into a
    capacity-C compacted buffer via dma_scatter_add (non-selected rows
    land in spread dump rows), then read back transposed per token
    group via dma_gather(transpose=True).
  - expert FFN runs dense on the C compacted tokens:
    hT = silu(W1n @ xgT + b1);  out = hT.T @ W2nT + b2.
  - outputs return to token order via dma_gather (dropped tokens read a
    zeroed dump row); one ReduceScatter sums the 8 expert partials and
    each core returns its T/8-row shard.
"""

import numpy as np

import concourse.bass as bass
import concourse.mybir as mybir
import concourse.tile as tile
from concourse import bacc
from concourse.library_config import mlp

F32 = mybir.dt.float32
BF16 = mybir.dt.bfloat16
I16 = mybir.dt.int16

AX = mybir.AxisListType
OP = mybir.AluOpType
ACT = mybir.ActivationFunctionType


class Cfg:
    def __init__(self, T=8192, D=1024, H=4096, E=8, NCORES=8, C=2304, S=256):
        self.T, self.D, self.H, self.E = T, D, H, E
        self.NCORES = NCORES
        self.C = C          # per-expert token capacity (multiple of S and 16)
        self.S = S          # GEMM token-group size (multiple of 128)
        self.TSL = T // NCORES   # tokens per core slice
        self.NCH = T // 128      # 128-token chunks
        self.DUMP = 64           # spread dump rows
        assert C % S == 0 and C % 16 == 0 and S % 128 == 0
        assert T % (NCORES * 128) == 0 and D % 128 == 0 and H % 128 == 0
        assert self.NCH <= 128


def build_moe(nc, cfg: Cfg):
    T, D, H, E, C, S = cfg.T, cfg.D, cfg.H, cfg.E, cfg.C, cfg.S
    TSL, NCH, DUMP = cfg.TSL, cfg.NCH, cfg.DUMP
    ND = D // 128   # D chunks of 128
    NH = H // 128   # H chunks of 128
    NDN = max(1, D // 512)       # GEMM2 output-column tiles
    DN = min(512, D)
    NCORES = cfg.NCORES
    WCH = 512                    # weight-norm square-accum column chunk

    # ---------------- kernel I/O ----------------
    xfull = nc.dram_tensor("xfull", [T, D], F32, kind="ExternalInput").ap()
    xslice = nc.dram_tensor("xslice", [TSL, D], F32, kind="ExternalInput").ap()
    gatev = nc.dram_tensor("gatev", [E, D], F32, kind="ExternalInput").ap()
    gateg = nc.dram_tensor("gateg", [E, 1], F32, kind="ExternalInput").ap()
    gateb = nc.dram_tensor("gateb", [1, E], F32, kind="ExternalInput").ap()
    w1v = nc.dram_tensor("w1v", [H, D], F32, kind="ExternalInput").ap()
    w1g = nc.dram_tensor("w1g", [128, NH], F32, kind="ExternalInput").ap()
    b1w = nc.dram_tensor("b1w", [128, NH], F32, kind="ExternalInput").ap()
    w2v = nc.dram_tensor("w2v", [D, H], F32, kind="ExternalInput").ap()
    w2g = nc.dram_tensor("w2g", [128, ND], F32, kind="ExternalInput").ap()
    b2rowb = nc.dram_tensor("b2rowb", [1, D], BF16, kind="ExternalInput").ap()
    u128 = nc.dram_tensor("u128", [128, 128], F32, kind="ExternalInput").ap()
    uNCH = nc.dram_tensor("uNCH", [NCH, NCH], F32, kind="ExternalInput").ap()
    ones1 = nc.dram_tensor("ones1", [1, 128], F32, kind="ExternalInput").ap()
    ones1b = nc.dram_tensor("ones1b", [1, 128], BF16, kind="ExternalInput").ap()
    ident = nc.dram_tensor("ident", [128, 128], F32, kind="ExternalInput").ap()
    identb = nc.dram_tensor("identb", [128, 128], BF16, kind="ExternalInput").ap()
    gidx = nc.dram_tensor("gidx", [128, C // 16], I16, kind="ExternalInput").ap()
    dumpx = nc.dram_tensor("dumpx", [128, 1], F32, kind="ExternalInput").ap()
    out_ext = nc.dram_tensor("out", [TSL, D], F32, kind="ExternalOutput").ap()

    # ---------------- internal DRAM ----------------
    xbf = nc.dram_tensor("xbf", [T, D], BF16).ap()
    xg = nc.dram_tensor("xg", [C + DUMP, D], BF16).ap()
    outg = nc.dram_tensor("outg", [C + DUMP, D], BF16).ap()
    a2a_in = nc.dram_tensor("a2a_in", [E * TSL], F32).ap()
    a2a_out = nc.dram_tensor("a2a_out", [E * TSL], F32).ap()
    posscr = nc.dram_tensor("posscr", [T], I16).ap()
    rs_in = nc.dram_tensor("rs_in", [T, D], BF16).ap()
    rs_out = nc.dram_tensor("rs_out", [TSL, D], BF16).ap()

    rg = [list(range(NCORES))]

    with tile.TileContext(nc) as tc:
        nc.gpsimd.load_library(mlp)

        # x -> bf16 staging copy (dram->dram cast DMA; overlaps with gating)
        nc.gpsimd.dma_start(out=xbf, in_=xfull)

        with (
            tc.tile_pool(name="consts", bufs=1) as cpool,
            tc.tile_pool(name="wbig", bufs=1) as wpool,
            tc.tile_pool(name="psum", bufs=2, space="PSUM") as psp,
        ):
            # ---- constants into SBUF ----
            u128_sb = cpool.tile([128, 128], F32)
            nc.sync.dma_start(u128_sb[:], u128)
            uN_sb = cpool.tile([NCH, NCH], F32)
            nc.sync.dma_start(uN_sb[:], uNCH)
            ones1_sb = cpool.tile([1, 128], F32)
            nc.sync.dma_start(ones1_sb[:], ones1)
            ones1b_sb = cpool.tile([1, 128], BF16)
            nc.sync.dma_start(ones1b_sb[:], ones1b)
            id_sb = cpool.tile([128, 128], F32)
            nc.sync.dma_start(id_sb[:], ident)
            idb_sb = cpool.tile([128, 128], BF16)
            nc.sync.dma_start(idb_sb[:], identb)
            gidx_sb = cpool.tile([128, C // 16], I16)
            nc.sync.dma_start(gidx_sb[:], gidx)
            dump_sb = cpool.tile([128, 1], F32)
            nc.sync.dma_start(dump_sb[:], dumpx)
            gateb_sb = cpool.tile([1, E], F32)
            nc.sync.dma_start(gateb_sb[:], gateb)
            w1g_sb = cpool.tile([128, NH], F32)
            nc.sync.dma_start(w1g_sb[:], w1g)
            b1w_sb = cpool.tile([128, NH], F32)
            nc.sync.dma_start(b1w_sb[:], b1w)
            w2g_sb = cpool.tile([128, ND], F32)
            nc.sync.dma_start(w2g_sb[:], w2g)
            b2b_sb = cpool.tile([1, D], BF16)
            nc.sync.dma_start(b2b_sb[:], b2rowb)
            gwT = cpool.tile([128, ND, E], F32)       # normalized gate W^T
            maskT = cpool.tile([E, TSL], F32)         # per-expert masks, my slice
            posw = cpool.tile([128, T // 16], I16)    # wrapped scatter positions
            w1T = wpool.tile([128, ND, H], BF16)      # [d, dc, h] = W1n^T
            w2T = wpool.tile([128, NH, D], BF16)      # [h, hc, d] = W2n^T

            def weight_norm_rows(pool, src_ap, nrows_tiles, ncols, g_sb, tag,
                                 stage_bufs):
                """yield (tile_idx, bf16-normalized [128, ncols] tile)."""
                nch = ncols // WCH if ncols >= WCH else 1
                cw = min(WCH, ncols)
                for r in range(nrows_tiles):
                    ws = pool.tile([128, ncols], F32, tag=f"{tag}s",
                                   bufs=stage_bufs)
                    nc.sync.dma_start(ws[:], src_ap[r * 128:(r + 1) * 128, :])
                    parts = pool.tile([128, nch], F32, tag=f"{tag}p")
                    scr = pool.tile([128, cw], F32, tag="wsq", bufs=1)
                    for cc in range(nch):
                        nc.scalar.activation(
                            scr[:], ws[:, cc * cw:(cc + 1) * cw], ACT.Square,
                            accum_out=parts[:, cc:cc + 1],
                        )
                    ssq = pool.tile([128, 1], F32, tag=f"{tag}ss")
                    nc.vector.tensor_reduce(ssq[:], parts[:], axis=AX.X, op=OP.add)
                    nc.scalar.sqrt(ssq[:], ssq[:])
                    nc.vector.tensor_scalar_max(ssq[:], ssq[:], 1e-12)
                    rc = pool.tile([128, 1], F32, tag=f"{tag}rc")
                    nc.vector.reciprocal(rc[:], ssq[:])
                    nc.vector.tensor_tensor(rc[:], rc[:], g_sb[:, r:r + 1],
                                            op=OP.mult)
                    wnb = pool.tile([128, ncols], BF16, tag=f"{tag}nb",
                                    bufs=stage_bufs)
                    nc.vector.tensor_scalar_mul(wnb[:], ws[:], rc[:])
                    yield r, wnb

            with tc.tile_pool(name="prep", bufs=2) as ppool:
                # gate weight-norm + transpose -> gwT [128d, ND, E]
                gv_sb = ppool.tile([E, D], F32, tag="gv")
                nc.sync.dma_start(gv_sb[:], gatev)
                gg_sb = ppool.tile([E, 1], F32, tag="gg")
                nc.sync.dma_start(gg_sb[:], gateg)
                gcw = min(WCH, D)
                gparts = ppool.tile([E, D // gcw], F32, tag="gparts")
                for cc in range(D // gcw):
                    gscr = ppool.tile([E, gcw], F32, tag="wsq", bufs=1)
                    nc.scalar.activation(gscr[:], gv_sb[:, cc * gcw:(cc + 1) * gcw],
                                         ACT.Square, accum_out=gparts[:, cc:cc + 1])
                gss = ppool.tile([E, 1], F32, tag="gss")
                nc.vector.tensor_reduce(gss[:], gparts[:], axis=AX.X, op=OP.add)
                nc.scalar.sqrt(gss[:], gss[:])
                nc.vector.tensor_scalar_max(gss[:], gss[:], 1e-12)
                grc = ppool.tile([E, 1], F32, tag="grc")
                nc.vector.reciprocal(grc[:], gss[:])
                nc.vector.tensor_tensor(grc[:], grc[:], gg_sb[:], op=OP.mult)
                gwn = ppool.tile([E, D], F32, tag="gwn")
                nc.vector.tensor_scalar_mul(gwn[:], gv_sb[:], grc[:])
                for dc in range(ND):
                    pt = psp.tile([128, 128], F32, tag="t128")
                    nc.tensor.transpose(pt[:, :E], gwn[:, dc * 128:(dc + 1) * 128],
                                        id_sb[:E, :E])
                    nc.vector.tensor_copy(gwT[:, dc, :], pt[:, :E])

                # expert weight prep
                for hc, wnb in weight_norm_rows(ppool, w1v, NH, D, w1g_sb, "w1", 2):
                    for dc in range(ND):
                        pt = psp.tile([128, 128], BF16, tag="t128b")
                        nc.tensor.transpose(pt[:], wnb[:, dc * 128:(dc + 1) * 128],
                                            idb_sb[:])
                        nc.vector.tensor_copy(w1T[:, dc, hc * 128:(hc + 1) * 128],
                                              pt[:])
                for dc, wnb in weight_norm_rows(ppool, w2v, ND, H, w2g_sb, "w2", 1):
                    for hc in range(NH):
                        pt = psp.tile([128, 128], BF16, tag="t128b")
                        nc.tensor.transpose(pt[:], wnb[:, hc * 128:(hc + 1) * 128],
                                            idb_sb[:])
                        nc.vector.tensor_copy(w2T[:, hc, dc * 128:(dc + 1) * 128],
                                              pt[:])

                # gating over my token slice -> maskT [E, TSL]
                for tb in range(TSL // 128):
                    xs = ppool.tile([128, D], F32, tag="gx", bufs=1)
                    nc.sync.dma_start(xs[:], xslice[tb * 128:(tb + 1) * 128, :])
                    xt = ppool.tile([128, ND, 128], F32, tag="gxt", bufs=1)
                    for dc in range(ND):
                        pt = psp.tile([128, 128], F32, tag="t128")
                        nc.tensor.transpose(pt[:], xs[:, dc * 128:(dc + 1) * 128],
                                            id_sb[:])
                        nc.vector.tensor_copy(xt[:, dc, :], pt[:])
                    pg = psp.tile([128, E], F32, tag="t128")
                    for dc in range(ND):
                        nc.tensor.matmul(
                            pg[:, :E], lhsT=xt[:, dc, :], rhs=gwT[:, dc, :],
                            start=(dc == 0), stop=False,
                        )
                    nc.tensor.matmul(
                        pg[:, :E], lhsT=ones1_sb[:], rhs=gateb_sb[:],
                        start=False, stop=True,
                    )
                    lg = ppool.tile([128, E], F32, tag="lg")
                    nc.vector.tensor_copy(lg[:], pg[:, :E])
                    mx1 = ppool.tile([128, 1], F32, tag="mx1")
                    nc.vector.tensor_reduce(mx1[:], lg[:], axis=AX.X, op=OP.max)
                    eq = ppool.tile([128, E], F32, tag="eq")
                    nc.vector.tensor_tensor(eq[:], lg[:],
                                            mx1[:].to_broadcast([128, E]),
                                            op=OP.is_equal)
                    nc.vector.tensor_scalar_mul(eq[:], eq[:], 1e30)
                    nc.vector.tensor_tensor(eq[:], lg[:], eq[:], op=OP.subtract)
                    mx2 = ppool.tile([128, 1], F32, tag="mx2")
                    nc.vector.tensor_reduce(mx2[:], eq[:], axis=AX.X, op=OP.max)
                    mk = ppool.tile([128, E], F32, tag="mk")
                    nc.vector.tensor_tensor(mk[:], lg[:],
                                            mx2[:].to_broadcast([128, E]),
                                            op=OP.is_ge)
                    pmt = psp.tile([128, 128], F32, tag="t128")
                    nc.tensor.transpose(pmt[:E, :], mk[:], id_sb[:])
                    nc.vector.tensor_copy(maskT[:, tb * 128:(tb + 1) * 128],
                                          pmt[:E, :])
            nc.sync.dma_start(a2a_in.rearrange("(a b) -> a b", a=E), maskT[:])
            nc.gpsimd.collective_compute(
                "AllToAll", OP.bypass, replica_groups=rg,
                ins=[a2a_in], outs=[a2a_out],
            )

            # ---- positions from my expert's full-T mask ----
            with tc.tile_pool(name="posp", bufs=2) as qpool:
                mask_sb = qpool.tile([128, NCH], F32, tag="maskf")
                nc.sync.dma_start(mask_sb[:],
                                  a2a_out.rearrange("(g p) -> p g", p=128))
                ppos = psp.tile([128, NCH], F32, tag="t128")
                nc.tensor.matmul(ppos[:], lhsT=u128_sb[:], rhs=mask_sb[:],
                                 start=True, stop=True)
                pref = qpool.tile([128, NCH], F32, tag="pref")
                nc.vector.tensor_copy(pref[:], ppos[:])
                prefT = psp.tile([128, 128], F32, tag="t128")
                nc.tensor.transpose(prefT[:NCH, :], pref[:], id_sb[:])
                tot = qpool.tile([NCH, 1], F32, tag="tot")
                nc.vector.tensor_copy(tot[:], prefT[:NCH, 127:128])
                poff = psp.tile([128, 128], F32, tag="t128")
                nc.tensor.matmul(poff[:NCH, :1], lhsT=uN_sb[:], rhs=tot[:],
                                 start=True, stop=True)
                offs = qpool.tile([NCH, 1], F32, tag="offs")
                nc.vector.tensor_copy(offs[:], poff[:NCH, :1])
                porow = psp.tile([128, 128], F32, tag="t128")
                nc.tensor.transpose(porow[:1, :NCH], offs[:], id_sb[:NCH, :NCH])
                offsrow = qpool.tile([1, NCH], F32, tag="offsrow")
                nc.vector.tensor_copy(offsrow[:], porow[:1, :NCH])
                pbc = psp.tile([128, NCH], F32, tag="t128")
                nc.tensor.matmul(pbc[:], lhsT=ones1_sb[:], rhs=offsrow[:],
                                 start=True, stop=True)
                pos = qpool.tile([128, NCH], F32, tag="pos")
                nc.vector.tensor_tensor(pos[:], pref[:], pbc[:], op=OP.add)
                nc.vector.tensor_tensor(pos[:], pos[:], mask_sb[:],
                                        op=OP.subtract)
                # select: pos if mask else dump row (C + p%DUMP); clamp for safety
                nc.vector.tensor_tensor(pos[:], pos[:],
                                        dump_sb[:].to_broadcast([128, NCH]),
                                        op=OP.subtract)
                nc.vector.tensor_tensor(pos[:], pos[:], mask_sb[:], op=OP.mult)
                nc.vector.tensor_tensor(pos[:], pos[:],
                                        dump_sb[:].to_broadcast([128, NCH]),
                                        op=OP.add)
                nc.vector.tensor_scalar_min(pos[:], pos[:], float(C + DUMP - 1))
                posi = qpool.tile([128, NCH], I16, tag="posi")
                nc.vector.tensor_copy(posi[:], pos[:])
                # wrap-shuffle through DRAM into [16, T/16] layout
                nc.sync.dma_start(posscr.rearrange("(g p) -> p g", p=128), posi[:])
                for r in range(8):
                    nc.sync.dma_start(posw[16 * r:16 * (r + 1), :],
                                      posscr.rearrange("(s q) -> q s", q=16))

                # zero compacted x buffer, then scatter x rows (bf16)
                zt = qpool.tile([128, D], BF16, tag="zero", bufs=1)
                nc.gpsimd.memset(zt[:], 0.0)
                for j in range(C // 128):
                    nc.sync.dma_start(xg[j * 128:(j + 1) * 128, :], zt[:])
                nc.sync.dma_start(xg[C:C + DUMP, :], zt[:DUMP, :])
                nc.sync.dma_start(outg[C:C + DUMP, :], zt[:DUMP, :])
                for b in range(T // 512):
                    xb = qpool.tile([128, 4, D], BF16, tag="xscat")
                    nc.sync.dma_start(
                        xb[:],
                        xbf[b * 512:(b + 1) * 512, :]
                        .rearrange("(a p) d -> p a d", p=128),
                    )
                    nc.gpsimd.dma_scatter_add(
                        xg, xb[:], posw[:, b * 32:(b + 1) * 32], 512, 512, D,
                    )

            # ---- expert FFN over compacted tokens ----
            with tc.tile_pool(name="gemm", bufs=2) as gpool:
                for g in range(C // S):
                    xgt = gpool.tile([128, ND, S], BF16, tag="xgt")
                    nc.gpsimd.dma_gather(
                        xgt[:], xg, gidx_sb[:, g * (S // 16):(g + 1) * (S // 16)],
                        S, S, D, transpose=True,
                    )
                    hT = gpool.tile([128, NH, S], BF16, tag="hT")
                    for hc in range(NH):
                        ph = psp.tile([128, S], F32, tag="ph")
                        for dc in range(ND):
                            nc.tensor.matmul(
                                ph[:], lhsT=w1T[:, dc, hc * 128:(hc + 1) * 128],
                                rhs=xgt[:, dc, :],
                                start=(dc == 0), stop=(dc == ND - 1),
                            )
                        hb = gpool.tile([128, S], F32, tag="hb")
                        nc.vector.tensor_scalar_add(hb[:], ph[:],
                                                    b1w_sb[:, hc:hc + 1])
                        sg = gpool.tile([128, S], F32, tag="sg")
                        nc.scalar.activation(sg[:], hb[:], ACT.Sigmoid)
                        nc.vector.tensor_tensor(hT[:, hc, :], hb[:], sg[:],
                                                op=OP.mult)
                    for tb in range(S // 128):
                        og = gpool.tile([128, D], BF16, tag="og")
                        for dn in range(NDN):
                            po = psp.tile([128, DN], F32, tag="po")
                            for hc in range(NH):
                                nc.tensor.matmul(
                                    po[:], lhsT=hT[:, hc, tb * 128:(tb + 1) * 128],
                                    rhs=w2T[:, hc, dn * DN:(dn + 1) * DN],
                                    start=(hc == 0), stop=False,
                                )
                            nc.tensor.matmul(
                                po[:], lhsT=ones1b_sb[:],
                                rhs=b2b_sb[:, dn * DN:(dn + 1) * DN],
                                start=False, stop=True,
                            )
                            nc.vector.tensor_copy(og[:, dn * DN:(dn + 1) * DN],
                                                  po[:])
                        nc.sync.dma_start(
                            outg[g * S + tb * 128:g * S + (tb + 1) * 128, :],
                            og[:],
                        )

            # ---- gather back to token order, ReduceScatter ----
            with tc.tile_pool(name="outp", bufs=2) as opool:
                for b in range(T // 1024):
                    rsb = opool.tile([128, 8, D], BF16, tag="rsb")
                    nc.gpsimd.dma_gather(
                        rsb[:], outg, posw[:, b * 64:(b + 1) * 64], 1024, 1024, D,
                    )
                    nc.sync.dma_start(
                        rs_in[b * 1024:(b + 1) * 1024, :]
                        .rearrange("(a p) d -> p a d", p=128),
                        rsb[:],
                    )
                nc.gpsimd.collective_compute(
                    "ReduceScatter", OP.add, replica_groups=rg,
                    ins=[rs_in], outs=[rs_out],
                )
                for j in range(TSL // 128):
                    rb = opool.tile([128, D], BF16, tag="finb")
                    nc.sync.dma_start(rb[:], rs_out[j * 128:(j + 1) * 128, :])
                    rf = opool.tile([128, D], F32, tag="finf")
                    nc.vector.tensor_copy(rf[:], rb[:])
                    nc.sync.dma_start(out_ext[j * 128:(j + 1) * 128, :], rf[:])

    return nc


def make_in_maps(cfg: Cfg, x, gate_v, gate_g, gate_b, w1_v, w1_g, b1, w2_v, w2_g, b2):
    """Build the per-core input maps from the full (unsharded) inputs."""
    import ml_dtypes

    T, D, H, E, C = cfg.T, cfg.D, cfg.H, cfg.E, cfg.C
    NH, ND, TSL, NCH = H // 128, D // 128, cfg.TSL, cfg.NCH
    f32 = np.float32
    xf = np.ascontiguousarray(x.reshape(T, D), dtype=f32)
    u128 = np.triu(np.ones((128, 128), f32))           # u[k,m]=1 iff k<=m
    uN = np.triu(np.ones((NCH, NCH), f32), 1)          # strict upper: k<m
    ones1 = np.ones((1, 128), f32)
    ident = np.eye(128, dtype=f32)
    gidx = np.ascontiguousarray(np.tile(
        np.arange(C, dtype=np.int16).reshape(C // 16, 16).T, (8, 1)))
    dumpx = (C + (np.arange(128) % cfg.DUMP)).astype(f32).reshape(128, 1)

    def bf16(a):
        return np.ascontiguousarray(a).astype(ml_dtypes.bfloat16)

    def wrap_pc(v, nch):  # [nch*128] -> [128, nch] with v[c*128+p] at [p, c]
        return np.ascontiguousarray(np.asarray(v, f32).reshape(nch, 128).T)

    in_maps = []
    for i in range(cfg.NCORES):
        in_maps.append({
            "xfull": xf,
            "xslice": np.ascontiguousarray(xf[i * TSL:(i + 1) * TSL]),
            "gatev": np.ascontiguousarray(gate_v, dtype=f32),
            "gateg": np.ascontiguousarray(np.asarray(gate_g, f32).reshape(E, 1)),
            "gateb": np.ascontiguousarray(np.asarray(gate_b, f32).reshape(1, E)),
            "w1v": np.ascontiguousarray(w1_v[i], dtype=f32),
            "w1g": wrap_pc(w1_g[i], NH),
            "b1w": wrap_pc(b1[i], NH),
            "w2v": np.ascontiguousarray(w2_v[i], dtype=f32),
            "w2g": wrap_pc(w2_g[i], ND),
            "b2rowb": bf16(np.asarray(b2[i], f32).reshape(1, D)),
            "u128": u128,
            "uNCH": uN,
            "ones1": ones1,
            "ones1b": bf16(ones1),
            "ident": ident,
            "identb": bf16(ident),
            "gidx": gidx,
            "dumpx": dumpx,
        })
    return in_maps


_COMPILED = {}


def get_compiled(cfg: Cfg):
    key = (cfg.T, cfg.D, cfg.H, cfg.E, cfg.C, cfg.S)
    if key not in _COMPILED:
        nc = bacc.Bacc("TRN2", target_bir_lowering=False, debug=False,
                       num_devices=cfg.NCORES)
        build_moe(nc, cfg)
        nc.compile()
        _COMPILED[key] = nc
    return _COMPILED[key]


def kernel(x, gate_v, gate_g, gate_b, w1_v, w1_g, b1, w2_v, w2_g, b2):
    from concourse.bass_utils import run_bass_kernel_spmd

    cfg = Cfg()
    nc = get_compiled(cfg)
    in_maps = make_in_maps(cfg, np.asarray(x), np.asarray(gate_v),
                           np.asarray(gate_g), np.asarray(gate_b),
                           np.asarray(w1_v), np.asarray(w1_g), np.asarray(b1),
                           np.asarray(w2_v), np.asarray(w2_g), np.asarray(b2))
    res = run_bass_kernel_spmd(nc, in_maps, core_ids=list(range(cfg.NCORES)))
    shards = [res.results[i]["out"] for i in range(cfg.NCORES)]
    out = np.concatenate(shards, axis=0).astype(np.float32)
    B, S_, D_ = x.shape
    return out.reshape(B, S_, D_)


# revision 12
# speedup vs baseline: 1.0278x; 1.0278x over previous
"""Expert-parallel MoE kernel for one TRN2 chip (8 NeuronCores).

Strategy (expert-parallel, top-2 sparse):
  - core e owns expert e's weights (weight-norm applied + transposed on
    device, bf16 compute / fp32 accumulate).
  - gating is data-parallel: core i computes top-2 masks for its T/8
    token slice of x with exact fp32 matmuls (so routing never flips),
    then an AllToAll exchanges per-expert mask rows so every core holds
    the full-T mask for ITS expert.
  - token compaction: matmul prefix-sums turn the mask into global
    compacted positions; x rows (bf16) are scatter-written into a
    capacity-C compacted buffer via dma_scatter_add (non-selected rows
    land in spread dump rows), then read back transposed per token
    group via dma_gather(transpose=True).
  - expert FFN runs dense on the C compacted tokens:
    hT = silu(W1n @ xgT + b1);  out = hT.T @ W2nT + b2.
  - outputs return to token order via dma_gather (dropped tokens read a
    zeroed dump row); one ReduceScatter sums the 8 expert partials and
    each core returns its T/8-row shard.
"""

import numpy as np

import concourse.bass as bass
import concourse.mybir as mybir
import concourse.tile as tile
from concourse import bacc
from concourse.library_config import mlp

F32 = mybir.dt.float32
BF16 = mybir.dt.bfloat16
I16 = mybir.dt.int16

AX = mybir.AxisListType
OP = mybir.AluOpType
ACT = mybir.ActivationFunctionType


class Cfg:
    def __init__(self, T=8192, D=1024, H=4096, E=8, NCORES=8, C=2304, S=256):
        self.T, self.D, self.H, self.E = T, D, H, E
        self.NCORES = NCORES
        self.C = C          # per-expert token capacity (multiple of S and 16)
        self.S = S          # GEMM token-group size (multiple of 128)
        self.TSL = T // NCORES   # tokens per core slice
        self.NCH = T // 128      # 128-token chunks
        self.DUMP = 64           # spread dump rows
        assert C % S == 0 and C % 16 == 0 and S % 128 == 0
        assert T % (NCORES * 128) == 0 and D % 128 == 0 and H % 128 == 0
        assert self.NCH <= 128


def build_moe(nc, cfg: Cfg):
    T, D, H, E, C, S = cfg.T, cfg.D, cfg.H, cfg.E, cfg.C, cfg.S
    TSL, NCH, DUMP = cfg.TSL, cfg.NCH, cfg.DUMP
    ND = D // 128   # D chunks of 128
    NH = H // 128   # H chunks of 128
    NDN = max(1, D // 512)       # GEMM2 output-column tiles
    DN = min(512, D)
    NCORES = cfg.NCORES
    WCH = 512                    # weight-norm square-accum column chunk

    # ---------------- kernel I/O ----------------
    xfull = nc.dram_tensor("xfull", [T, D], F32, kind="ExternalInput").ap()
    xslice = nc.dram_tensor("xslice", [TSL, D], F32, kind="ExternalInput").ap()
    gatev = nc.dram_tensor("gatev", [E, D], F32, kind="ExternalInput").ap()
    gateg = nc.dram_tensor("gateg", [E, 1], F32, kind="ExternalInput").ap()
    gateb = nc.dram_tensor("gateb", [1, E], F32, kind="ExternalInput").ap()
    w1v = nc.dram_tensor("w1v", [H, D], F32, kind="ExternalInput").ap()
    w1g = nc.dram_tensor("w1g", [128, NH], F32, kind="ExternalInput").ap()
    b1w = nc.dram_tensor("b1w", [128, NH], F32, kind="ExternalInput").ap()
    w2v = nc.dram_tensor("w2v", [D, H], F32, kind="ExternalInput").ap()
    w2g = nc.dram_tensor("w2g", [128, ND], F32, kind="ExternalInput").ap()
    b2rowb = nc.dram_tensor("b2rowb", [1, D], BF16, kind="ExternalInput").ap()
    u128 = nc.dram_tensor("u128", [128, 128], F32, kind="ExternalInput").ap()
    uNCH = nc.dram_tensor("uNCH", [NCH, NCH], F32, kind="ExternalInput").ap()
    ones1 = nc.dram_tensor("ones1", [1, 128], F32, kind="ExternalInput").ap()
    ones1b = nc.dram_tensor("ones1b", [1, 128], BF16, kind="ExternalInput").ap()
    ident = nc.dram_tensor("ident", [128, 128], F32, kind="ExternalInput").ap()
    identb = nc.dram_tensor("identb", [128, 128], BF16, kind="ExternalInput").ap()
    gidx = nc.dram_tensor("gidx", [128, C // 16], I16, kind="ExternalInput").ap()
    dumpx = nc.dram_tensor("dumpx", [128, 1], F32, kind="ExternalInput").ap()
    out_ext = nc.dram_tensor("out", [TSL, D], F32, kind="ExternalOutput").ap()

    # ---------------- internal DRAM ----------------
    xbf = nc.dram_tensor("xbf", [T, D], BF16).ap()
    xg = nc.dram_tensor("xg", [C + DUMP, D], BF16).ap()
    outg = nc.dram_tensor("outg", [C + DUMP, D], BF16).ap()
    a2a_in = nc.dram_tensor("a2a_in", [E * TSL], F32).ap()
    a2a_out = nc.dram_tensor("a2a_out", [E * TSL], F32).ap()
    rs_in = nc.dram_tensor("rs_in", [T, D], BF16).ap()
    rs_out = nc.dram_tensor("rs_out", [TSL, D], BF16).ap()

    rg = [list(range(NCORES))]

    with tile.TileContext(nc) as tc:
        nc.gpsimd.load_library(mlp)

        # x -> bf16 staging copy (dram->dram cast DMA; overlaps with gating)
        nc.gpsimd.dma_start(out=xbf, in_=xfull)

        with (
            tc.tile_pool(name="consts", bufs=1) as cpool,
            tc.tile_pool(name="wbig", bufs=1) as wpool,
            tc.tile_pool(name="psum", bufs=2, space="PSUM") as psp,
        ):
            # ---- constants into SBUF ----
            u128_sb = cpool.tile([128, 128], F32)
            nc.sync.dma_start(u128_sb[:], u128)
            uN_sb = cpool.tile([NCH, NCH], F32)
            nc.sync.dma_start(uN_sb[:], uNCH)
            ones1_sb = cpool.tile([1, 128], F32)
            nc.sync.dma_start(ones1_sb[:], ones1)
            ones1b_sb = cpool.tile([1, 128], BF16)
            nc.sync.dma_start(ones1b_sb[:], ones1b)
            id_sb = cpool.tile([128, 128], F32)
            nc.sync.dma_start(id_sb[:], ident)
            idb_sb = cpool.tile([128, 128], BF16)
            nc.sync.dma_start(idb_sb[:], identb)
            gidx_sb = cpool.tile([128, C // 16], I16)
            nc.sync.dma_start(gidx_sb[:], gidx)
            dump_sb = cpool.tile([128, 1], F32)
            nc.sync.dma_start(dump_sb[:], dumpx)
            gateb_sb = cpool.tile([1, E], F32)
            nc.sync.dma_start(gateb_sb[:], gateb)
            w1g_sb = cpool.tile([128, NH], F32)
            nc.sync.dma_start(w1g_sb[:], w1g)
            b1w_sb = cpool.tile([128, NH], F32)
            nc.sync.dma_start(b1w_sb[:], b1w)
            w2g_sb = cpool.tile([128, ND], F32)
            nc.sync.dma_start(w2g_sb[:], w2g)
            b2b_sb = cpool.tile([1, D], BF16)
            nc.sync.dma_start(b2b_sb[:], b2rowb)
            gwT = cpool.tile([128, ND, E], F32)       # normalized gate W^T
            maskT = cpool.tile([E, TSL], F32)         # per-expert masks, my slice
            posw = cpool.tile([128, T // 16], I16)    # wrapped scatter positions
            w1T = wpool.tile([128, ND, H], BF16)      # [d, dc, h] = W1n^T
            w2T = wpool.tile([128, NH, D], BF16)      # [h, hc, d] = W2n^T

            def weight_norm_rows(pool, src_ap, nrows_tiles, ncols, g_sb, tag,
                                 stage_bufs):
                """yield (tile_idx, bf16-normalized [128, ncols] tile)."""
                nch = ncols // WCH if ncols >= WCH else 1
                cw = min(WCH, ncols)
                for r in range(nrows_tiles):
                    ws = pool.tile([128, ncols], F32, tag=f"{tag}s",
                                   bufs=stage_bufs)
                    nc.sync.dma_start(ws[:], src_ap[r * 128:(r + 1) * 128, :])
                    parts = pool.tile([128, nch], F32, tag=f"{tag}p")
                    scr = pool.tile([128, cw], F32, tag="wsq", bufs=1)
                    for cc in range(nch):
                        nc.scalar.activation(
                            scr[:], ws[:, cc * cw:(cc + 1) * cw], ACT.Square,
                            accum_out=parts[:, cc:cc + 1],
                        )
                    ssq = pool.tile([128, 1], F32, tag=f"{tag}ss")
                    nc.vector.tensor_reduce(ssq[:], parts[:], axis=AX.X, op=OP.add)
                    nc.scalar.sqrt(ssq[:], ssq[:])
                    nc.vector.tensor_scalar_max(ssq[:], ssq[:], 1e-12)
                    rc = pool.tile([128, 1], F32, tag=f"{tag}rc")
                    nc.vector.reciprocal(rc[:], ssq[:])
                    nc.vector.tensor_tensor(rc[:], rc[:], g_sb[:, r:r + 1],
                                            op=OP.mult)
                    wnb = pool.tile([128, ncols], BF16, tag=f"{tag}nb",
                                    bufs=stage_bufs)
                    nc.vector.tensor_scalar_mul(wnb[:], ws[:], rc[:])
                    yield r, wnb

            with tc.tile_pool(name="prep", bufs=2) as ppool:
                # gate weight-norm + transpose -> gwT [128d, ND, E]
                gv_sb = ppool.tile([E, D], F32, tag="gv")
                nc.sync.dma_start(gv_sb[:], gatev)
                gg_sb = ppool.tile([E, 1], F32, tag="gg")
                nc.sync.dma_start(gg_sb[:], gateg)
                gcw = min(WCH, D)
                gparts = ppool.tile([E, D // gcw], F32, tag="gparts")
                for cc in range(D // gcw):
                    gscr = ppool.tile([E, gcw], F32, tag="wsq", bufs=1)
                    nc.scalar.activation(gscr[:], gv_sb[:, cc * gcw:(cc + 1) * gcw],
                                         ACT.Square, accum_out=gparts[:, cc:cc + 1])
                gss = ppool.tile([E, 1], F32, tag="gss")
                nc.vector.tensor_reduce(gss[:], gparts[:], axis=AX.X, op=OP.add)
                nc.scalar.sqrt(gss[:], gss[:])
                nc.vector.tensor_scalar_max(gss[:], gss[:], 1e-12)
                grc = ppool.tile([E, 1], F32, tag="grc")
                nc.vector.reciprocal(grc[:], gss[:])
                nc.vector.tensor_tensor(grc[:], grc[:], gg_sb[:], op=OP.mult)
                gwn = ppool.tile([E, D], F32, tag="gwn")
                nc.vector.tensor_scalar_mul(gwn[:], gv_sb[:], grc[:])
                for dc in range(ND):
                    pt = psp.tile([128, 128], F32, tag="t128")
                    nc.tensor.transpose(pt[:, :E], gwn[:, dc * 128:(dc + 1) * 128],
                                        id_sb[:E, :E])
                    nc.vector.tensor_copy(gwT[:, dc, :], pt[:, :E])

                # expert weight prep
                for hc, wnb in weight_norm_rows(ppool, w1v, NH, D, w1g_sb, "w1", 2):
                    for dc in range(ND):
                        pt = psp.tile([128, 128], BF16, tag="t128b")
                        nc.tensor.transpose(pt[:], wnb[:, dc * 128:(dc + 1) * 128],
                                            idb_sb[:])
                        nc.vector.tensor_copy(w1T[:, dc, hc * 128:(hc + 1) * 128],
                                              pt[:])
                for dc, wnb in weight_norm_rows(ppool, w2v, ND, H, w2g_sb, "w2", 1):
                    for hc in range(NH):
                        pt = psp.tile([128, 128], BF16, tag="t128b")
                        nc.tensor.transpose(pt[:], wnb[:, hc * 128:(hc + 1) * 128],
                                            idb_sb[:])
                        nc.vector.tensor_copy(w2T[:, hc, dc * 128:(dc + 1) * 128],
                                              pt[:])

                # gating over my token slice -> maskT [E, TSL]
                for tb in range(TSL // 128):
                    xs = ppool.tile([128, D], F32, tag="gx", bufs=1)
                    nc.sync.dma_start(xs[:], xslice[tb * 128:(tb + 1) * 128, :])
                    xt = ppool.tile([128, ND, 128], F32, tag="gxt", bufs=1)
                    for dc in range(ND):
                        pt = psp.tile([128, 128], F32, tag="t128")
                        nc.tensor.transpose(pt[:], xs[:, dc * 128:(dc + 1) * 128],
                                            id_sb[:])
                        nc.vector.tensor_copy(xt[:, dc, :], pt[:])
                    pg = psp.tile([128, E], F32, tag="t128")
                    for dc in range(ND):
                        nc.tensor.matmul(
                            pg[:, :E], lhsT=xt[:, dc, :], rhs=gwT[:, dc, :],
                            start=(dc == 0), stop=False,
                        )
                    nc.tensor.matmul(
                        pg[:, :E], lhsT=ones1_sb[:], rhs=gateb_sb[:],
                        start=False, stop=True,
                    )
                    lg = ppool.tile([128, E], F32, tag="lg")
                    nc.vector.tensor_copy(lg[:], pg[:, :E])
                    mx1 = ppool.tile([128, 1], F32, tag="mx1")
                    nc.vector.tensor_reduce(mx1[:], lg[:], axis=AX.X, op=OP.max)
                    eq = ppool.tile([128, E], F32, tag="eq")
                    nc.vector.tensor_tensor(eq[:], lg[:],
                                            mx1[:].to_broadcast([128, E]),
                                            op=OP.is_equal)
                    nc.vector.tensor_scalar_mul(eq[:], eq[:], 1e30)
                    nc.vector.tensor_tensor(eq[:], lg[:], eq[:], op=OP.subtract)
                    mx2 = ppool.tile([128, 1], F32, tag="mx2")
                    nc.vector.tensor_reduce(mx2[:], eq[:], axis=AX.X, op=OP.max)
                    mk = ppool.tile([128, E], F32, tag="mk")
                    nc.vector.tensor_tensor(mk[:], lg[:],
                                            mx2[:].to_broadcast([128, E]),
                                            op=OP.is_ge)
                    pmt = psp.tile([128, 128], F32, tag="t128")
                    nc.tensor.transpose(pmt[:E, :], mk[:], id_sb[:])
                    nc.vector.tensor_copy(maskT[:, tb * 128:(tb + 1) * 128],
                                          pmt[:E, :])
            nc.sync.dma_start(a2a_in.rearrange("(a b) -> a b", a=E), maskT[:])
            nc.gpsimd.collective_compute(
                "AllToAll", OP.bypass, replica_groups=rg,
                ins=[a2a_in], outs=[a2a_out],
            )

            # ---- positions from my expert's full-T mask ----
            with tc.tile_pool(name="posp", bufs=2) as qpool:
                mrow = qpool.tile([NCH, 128], F32, tag="mrow")
                nc.sync.dma_start(mrow[:],
                                  a2a_out.rearrange("(g p) -> g p", p=128))
                pmk = psp.tile([128, 128], F32, tag="t128")
                nc.tensor.transpose(pmk[:, :NCH], mrow[:], id_sb[:NCH, :NCH])
                mask_sb = qpool.tile([128, NCH], F32, tag="maskf")
                nc.vector.tensor_copy(mask_sb[:], pmk[:, :NCH])
                ppos = psp.tile([128, NCH], F32, tag="t128")
                nc.tensor.matmul(ppos[:], lhsT=u128_sb[:], rhs=mask_sb[:],
                                 start=True, stop=True)
                pref = qpool.tile([128, NCH], F32, tag="pref")
                nc.vector.tensor_copy(pref[:], ppos[:])
                prefT = psp.tile([128, 128], F32, tag="t128")
                nc.tensor.transpose(prefT[:NCH, :], pref[:], id_sb[:])
                tot = qpool.tile([NCH, 1], F32, tag="tot")
                nc.vector.tensor_copy(tot[:], prefT[:NCH, 127:128])
                poff = psp.tile([128, 128], F32, tag="t128")
                nc.tensor.matmul(poff[:NCH, :1], lhsT=uN_sb[:], rhs=tot[:],
                                 start=True, stop=True)
                offs = qpool.tile([NCH, 1], F32, tag="offs")
                nc.vector.tensor_copy(offs[:], poff[:NCH, :1])
                porow = psp.tile([128, 128], F32, tag="t128")
                nc.tensor.transpose(porow[:1, :NCH], offs[:], id_sb[:NCH, :NCH])
                offsrow = qpool.tile([1, NCH], F32, tag="offsrow")
                nc.vector.tensor_copy(offsrow[:], porow[:1, :NCH])
                pbc = psp.tile([128, NCH], F32, tag="t128")
                nc.tensor.matmul(pbc[:], lhsT=ones1_sb[:], rhs=offsrow[:],
                                 start=True, stop=True)
                pos = qpool.tile([128, NCH], F32, tag="pos")
                nc.vector.tensor_tensor(pos[:], pref[:], pbc[:], op=OP.add)
                nc.vector.tensor_tensor(pos[:], pos[:], mask_sb[:],
                                        op=OP.subtract)
                # select: pos if mask else dump row (C + p%DUMP); clamp for safety
                nc.vector.tensor_tensor(pos[:], pos[:],
                                        dump_sb[:].to_broadcast([128, NCH]),
                                        op=OP.subtract)
                nc.vector.tensor_tensor(pos[:], pos[:], mask_sb[:], op=OP.mult)
                nc.vector.tensor_tensor(pos[:], pos[:],
                                        dump_sb[:].to_broadcast([128, NCH]),
                                        op=OP.add)
                nc.vector.tensor_scalar_min(pos[:], pos[:], float(C + DUMP - 1))
                # on-chip wrap shuffle: [p, g] -> [q, (g, ph)] with p = ph*16+q
                pT = psp.tile([128, 128], F32, tag="t128")
                nc.tensor.transpose(pT[:NCH, :], pos[:], id_sb[:])
                posTs = qpool.tile([NCH, 128], F32, tag="posTs")
                nc.vector.tensor_copy(posTs[:], pT[:NCH, :])
                poswf = qpool.tile([16, NCH, 8], F32, tag="poswf")
                for ph in range(8):
                    pq = psp.tile([128, 128], F32, tag="t128")
                    nc.tensor.transpose(pq[:16, :NCH],
                                        posTs[:, ph * 16:(ph + 1) * 16],
                                        id_sb[:NCH, :NCH])
                    nc.vector.tensor_copy(poswf[:, :, ph], pq[:16, :NCH])
                posw16 = qpool.tile([16, T // 16], I16, tag="posw16")
                nc.vector.tensor_copy(posw16[:],
                                      poswf[:].rearrange("q g h -> q (g h)"))
                nc.sync.dma_start(posw[0:16, :], posw16[:])
                nc.sync.dma_start(posw[16:32, :], posw[0:16, :])
                nc.sync.dma_start(posw[32:64, :], posw[0:32, :])
                nc.sync.dma_start(posw[64:128, :], posw[0:64, :])

                # zero compacted x buffer, then scatter x rows (bf16)
                zt = qpool.tile([128, D], BF16, tag="zero", bufs=1)
                nc.gpsimd.memset(zt[:], 0.0)
                for j in range(C // 128):
                    nc.sync.dma_start(xg[j * 128:(j + 1) * 128, :], zt[:])
                nc.sync.dma_start(xg[C:C + DUMP, :], zt[:DUMP, :])
                nc.sync.dma_start(outg[C:C + DUMP, :], zt[:DUMP, :])
                for b in range(T // 512):
                    xb = qpool.tile([128, 4, D], BF16, tag="xscat")
                    nc.sync.dma_start(
                        xb[:],
                        xbf[b * 512:(b + 1) * 512, :]
                        .rearrange("(a p) d -> p a d", p=128),
                    )
                    nc.gpsimd.dma_scatter_add(
                        xg, xb[:], posw[:, b * 32:(b + 1) * 32], 512, 512, D,
                    )

            # ---- expert FFN over compacted tokens ----
            with tc.tile_pool(name="gemm", bufs=2) as gpool:
                for g in range(C // S):
                    xgt = gpool.tile([128, ND, S], BF16, tag="xgt")
                    nc.gpsimd.dma_gather(
                        xgt[:], xg, gidx_sb[:, g * (S // 16):(g + 1) * (S // 16)],
                        S, S, D, transpose=True,
                    )
                    hT = gpool.tile([128, NH, S], BF16, tag="hT")
                    for hc in range(NH):
                        ph = psp.tile([128, S], F32, tag="ph")
                        for dc in range(ND):
                            nc.tensor.matmul(
                                ph[:], lhsT=w1T[:, dc, hc * 128:(hc + 1) * 128],
                                rhs=xgt[:, dc, :],
                                start=(dc == 0), stop=(dc == ND - 1),
                            )
                        hb = gpool.tile([128, S], F32, tag="hb")
                        nc.vector.tensor_scalar_add(hb[:], ph[:],
                                                    b1w_sb[:, hc:hc + 1])
                        sg = gpool.tile([128, S], F32, tag="sg")
                        nc.scalar.activation(sg[:], hb[:], ACT.Sigmoid)
                        nc.vector.tensor_tensor(hT[:, hc, :], hb[:], sg[:],
                                                op=OP.mult)
                    for tb in range(S // 128):
                        og = gpool.tile([128, D], BF16, tag="og")
                        for dn in range(NDN):
                            po = psp.tile([128, DN], F32, tag="po")
                            for hc in range(NH):
                                nc.tensor.matmul(
                                    po[:], lhsT=hT[:, hc, tb * 128:(tb + 1) * 128],
                                    rhs=w2T[:, hc, dn * DN:(dn + 1) * DN],
                                    start=(hc == 0), stop=False,
                                )
                            nc.tensor.matmul(
                                po[:], lhsT=ones1b_sb[:],
                                rhs=b2b_sb[:, dn * DN:(dn + 1) * DN],
                                start=False, stop=True,
                            )
                            nc.vector.tensor_copy(og[:, dn * DN:(dn + 1) * DN],
                                                  po[:])
                        nc.sync.dma_start(
                            outg[g * S + tb * 128:g * S + (tb + 1) * 128, :],
                            og[:],
                        )

            # ---- gather back to token order, ReduceScatter ----
            with tc.tile_pool(name="outp", bufs=2) as opool:
                for b in range(T // 1024):
                    rsb = opool.tile([128, 8, D], BF16, tag="rsb")
                    nc.gpsimd.dma_gather(
                        rsb[:], outg, posw[:, b * 64:(b + 1) * 64], 1024, 1024, D,
                    )
                    nc.sync.dma_start(
                        rs_in[b * 1024:(b + 1) * 1024, :]
                        .rearrange("(a p) d -> p a d", p=128),
                        rsb[:],
                    )
                nc.gpsimd.collective_compute(
                    "ReduceScatter", OP.add, replica_groups=rg,
                    ins=[rs_in], outs=[rs_out],
                )
                for j in range(TSL // 128):
                    rb = opool.tile([128, D], BF16, tag="finb")
                    nc.sync.dma_start(rb[:], rs_out[j * 128:(j + 1) * 128, :])
                    rf = opool.tile([128, D], F32, tag="finf")
                    nc.vector.tensor_copy(rf[:], rb[:])
                    nc.sync.dma_start(out_ext[j * 128:(j + 1) * 128, :], rf[:])

    return nc


def make_in_maps(cfg: Cfg, x, gate_v, gate_g, gate_b, w1_v, w1_g, b1, w2_v, w2_g, b2):
    """Build the per-core input maps from the full (unsharded) inputs."""
    import ml_dtypes

    T, D, H, E, C = cfg.T, cfg.D, cfg.H, cfg.E, cfg.C
    NH, ND, TSL, NCH = H // 128, D // 128, cfg.TSL, cfg.NCH
    f32 = np.float32
    xf = np.ascontiguousarray(x.reshape(T, D), dtype=f32)
    u128 = np.triu(np.ones((128, 128), f32))           # u[k,m]=1 iff k<=m
    uN = np.triu(np.ones((NCH, NCH), f32), 1)          # strict upper: k<m
    ones1 = np.ones((1, 128), f32)
    ident = np.eye(128, dtype=f32)
    gidx = np.ascontiguousarray(np.tile(
        np.arange(C, dtype=np.int16).reshape(C // 16, 16).T, (8, 1)))
    dumpx = (C + (np.arange(128) % cfg.DUMP)).astype(f32).reshape(128, 1)

    def bf16(a):
        return np.ascontiguousarray(a).astype(ml_dtypes.bfloat16)

    def wrap_pc(v, nch):  # [nch*128] -> [128, nch] with v[c*128+p] at [p, c]
        return np.ascontiguousarray(np.asarray(v, f32).reshape(nch, 128).T)

    in_maps = []
    for i in range(cfg.NCORES):
        in_maps.append({
            "xfull": xf,
            "xslice": np.ascontiguousarray(xf[i * TSL:(i + 1) * TSL]),
            "gatev": np.ascontiguousarray(gate_v, dtype=f32),
            "gateg": np.ascontiguousarray(np.asarray(gate_g, f32).reshape(E, 1)),
            "gateb": np.ascontiguousarray(np.asarray(gate_b, f32).reshape(1, E)),
            "w1v": np.ascontiguousarray(w1_v[i], dtype=f32),
            "w1g": wrap_pc(w1_g[i], NH),
            "b1w": wrap_pc(b1[i], NH),
            "w2v": np.ascontiguousarray(w2_v[i], dtype=f32),
            "w2g": wrap_pc(w2_g[i], ND),
            "b2rowb": bf16(np.asarray(b2[i], f32).reshape(1, D)),
            "u128": u128,
            "uNCH": uN,
            "ones1": ones1,
            "ones1b": bf16(ones1),
            "ident": ident,
            "identb": bf16(ident),
            "gidx": gidx,
            "dumpx": dumpx,
        })
    return in_maps


_COMPILED = {}


def get_compiled(cfg: Cfg):
    key = (cfg.T, cfg.D, cfg.H, cfg.E, cfg.C, cfg.S)
    if key not in _COMPILED:
        nc = bacc.Bacc("TRN2", target_bir_lowering=False, debug=False,
                       num_devices=cfg.NCORES)
        build_moe(nc, cfg)
        nc.compile()
        _COMPILED[key] = nc
    return _COMPILED[key]


def kernel(x, gate_v, gate_g, gate_b, w1_v, w1_g, b1, w2_v, w2_g, b2):
    from concourse.bass_utils import run_bass_kernel_spmd

    cfg = Cfg()
    nc = get_compiled(cfg)
    in_maps = make_in_maps(cfg, np.asarray(x), np.asarray(gate_v),
                           np.asarray(gate_g), np.asarray(gate_b),
                           np.asarray(w1_v), np.asarray(w1_g), np.asarray(b1),
                           np.asarray(w2_v), np.asarray(w2_g), np.asarray(b2))
    res = run_bass_kernel_spmd(nc, in_maps, core_ids=list(range(cfg.NCORES)))
    shards = [res.results[i]["out"] for i in range(cfg.NCORES)]
    out = np.concatenate(shards, axis=0).astype(np.float32)
    B, S_, D_ = x.shape
    return out.reshape(B, S_, D_)
